# revision 2
# baseline (speedup 1.0000x reference)
"""Trainium2 Bass kernel for nn_Attention_54717883351680.

Math: with HEADS=1 the softmax in the reference is over a size-1 axis, so
attn == 1 and the whole module collapses to

    out[0, i, j, :] = v[i] * W_out[:, 0] + b_out        (independent of j)
    v[i] = x[0, i, :] @ W_qkv[2, :]

The problem is pure memory bandwidth: write 256 MB of broadcast rows.
Sharding: queries i are split across 8 cores (256 rows each -> 32 MB/core).
Each core computes its (256, 16) row table on-chip, replicates it along the
free axis in SBUF, and streams its contiguous 32 MB output shard to DRAM
with large DMAs whose source APs use stride-0 (broadcast) repeats.
"""

import numpy as np

import concourse.bass as bass
import concourse.mybir as mybir
from concourse.bass_utils import run_bass_kernel_spmd

# Problem shape (hardcoded; kernel.py must be self-contained).
B, L, DIM = 1, 2048, 16
N_CORES = 8
QS = L // N_CORES          # queries per core = 256
P = 128                    # SBUF partitions
G = QS // P                # query groups per core = 2
J0 = 512                   # j-replication materialized in SBUF
R = L // J0                # stride-0 repeats per output DMA
E = J0 * DIM               # free elems per (partition, rep) = 8192
F32 = mybir.dt.float32

_cache = {}


def _build_nc():
    nc = bass.Bass()
    xs = nc.declare_dram_parameter("xs", [QS, DIM], F32, isOutput=False)
    cs = nc.declare_dram_parameter("consts", [3, DIM], F32, isOutput=False)
    out = nc.declare_dram_parameter("out", [QS, L * DIM], F32, isOutput=True)

    with (
        nc.sbuf_tensor([P, G * DIM], F32) as xsb,     # [p, g*16+d] = x[g*128+p, d]
        nc.sbuf_tensor([P, 3 * DIM], F32) as csb,     # wv | wout | bout, bcast to all p
        nc.sbuf_tensor([P, G * DIM], F32) as prod,
        nc.sbuf_tensor([P, G], F32) as vsb,           # v[p, g]
        nc.sbuf_tensor([P, G * DIM], F32) as rowsb,   # row[p, g*16+d]
        nc.sbuf_tensor([P, E], F32) as rep0,
        nc.sbuf_tensor([P, E], F32) as rep1,
        nc.semaphore() as dsem,
        nc.semaphore() as vsem,
        nc.semaphore() as osem,
        nc.Block() as block,
    ):
        reps = (rep0, rep1)

        @block.sync
        def _(sync):
            sync.dma_start(
                xsb[:].rearrange("p (g d) -> p g d", d=DIM),
                xs[:].rearrange("(g p) d -> p g d", p=P),
            ).then_inc(dsem, 16)
            for g in range(G):
                rep = reps[g]
                sync.wait_ge(vsem, 6 + g)  # rep_g copy retired
                sync.dma_start(
                    out[:][g * P:(g + 1) * P, :].rearrange("p (r e) -> p r e", e=E),
                    rep[:][:, None, :].broadcast_to((P, R, E)),
                ).then_inc(osem, 16)
            sync.wait_ge(osem, 16 * G)

        @block.scalar
        def _(scalar):
            scalar.dma_start(
                csb[:].rearrange("p (k d) -> p k d", d=DIM),
                cs[:][None, :, :].broadcast_to((P, 3, DIM)),
            ).then_inc(dsem, 16)

        @block.vector
        def _(vector):
            # DVE is deeply pipelined with no SBUF scoreboard: every RAW
            # between instructions needs a semaphore edge, even same-engine.
            vector.wait_ge(dsem, 32)
            wv = csb[:][:, 0:DIM][:, None, :].broadcast_to((P, G, DIM))
            wout = csb[:][:, DIM:2 * DIM]
            bout = csb[:][:, 2 * DIM:3 * DIM][:, None, :].broadcast_to((P, G, DIM))
            x3 = xsb[:].rearrange("p (g d) -> p g d", d=DIM)
            vector.tensor_mul(
                prod[:].rearrange("p (g d) -> p g d", d=DIM), x3, wv
            ).then_inc(vsem, 1)
            vector.wait_ge(vsem, 1)
            vector.tensor_reduce(
                vsb[:], prod[:].rearrange("p (g d) -> p g d", d=DIM),
                axis=mybir.AxisListType.X, op=mybir.AluOpType.add,
            ).then_inc(vsem, 1)
            vector.wait_ge(vsem, 2)
            for g in range(G):
                vector.tensor_mul(
                    rowsb[:, g * DIM:(g + 1) * DIM], wout,
                    vsb[:][:, g:g + 1].to_broadcast((P, DIM)),
                ).then_inc(vsem, 1)
            vector.wait_ge(vsem, 4)
            vector.tensor_add(
                rowsb[:].rearrange("p (g d) -> p g d", d=DIM),
                rowsb[:].rearrange("p (g d) -> p g d", d=DIM), bout,
            ).then_inc(vsem, 1)
            vector.wait_ge(vsem, 5)
            for g in range(G):
                vector.tensor_copy(
                    reps[g][:].rearrange("p (r d) -> p r d", d=DIM),
                    rowsb[:][:, g * DIM:(g + 1) * DIM][:, None, :]
                    .broadcast_to((P, J0, DIM)),
                ).then_inc(vsem, 1)

    return nc


def _get_nc():
    if "nc" not in _cache:
        _cache["nc"] = _build_nc()
    return _cache["nc"]


def run(x, W_qkv, W_out, b_out, trace=False):
    nc = _get_nc()
    consts = np.ascontiguousarray(
        np.stack([W_qkv[2, :], W_out[:, 0], b_out]).astype(np.float32)
    )
    in_maps = [
        {
            "xs": np.ascontiguousarray(x[0, c * QS:(c + 1) * QS, :], dtype=np.float32),
            "consts": consts,
        }
        for c in range(N_CORES)
    ]
    res = run_bass_kernel_spmd(nc, in_maps, list(range(N_CORES)), trace=trace)
    shards = [res.results[c]["out"].reshape(QS, L, DIM) for c in range(N_CORES)]
    full = np.concatenate(shards, axis=0)[None]  # (1, 2048, 2048, 16)
    return full, res.exec_time_ns


def kernel(x, W_qkv, W_out, b_out):
    out, _ = run(x, W_qkv, W_out, b_out, trace=False)
    return out


# revision 3
# speedup vs baseline: 1.0097x; 1.0097x over previous
"""Trainium2 Bass kernel for nn_Attention_54717883351680.

Math: with HEADS=1 the softmax in the reference is over a size-1 axis, so
attn == 1 and the whole module collapses to

    out[0, i, j, :] = v[i] * W_out[:, 0] + b_out        (independent of j)
    v[i] = x[0, i, :] @ W_qkv[2, :]

The problem is pure memory bandwidth: write 256 MB of broadcast rows.
Sharding: queries i are split across 8 cores (256 rows each -> 32 MB/core).
Each core computes v for its queries on-chip, writes the replicated row
blocks into two small SBUF tiles (one per 128-query group), and streams its
contiguous 32 MB output shard to DRAM with one large DMA per group whose
source AP uses stride-0 (broadcast) repeats of the tile.
"""

import numpy as np

import concourse.bass as bass
import concourse.mybir as mybir
from concourse.bass_utils import run_bass_kernel_spmd

# Problem shape (hardcoded; kernel.py must be self-contained).
B, L, DIM = 1, 2048, 16
N_CORES = 8
QS = L // N_CORES          # queries per core = 256
P = 128                    # SBUF partitions
G = QS // P                # query groups per core = 2
J0 = 128                   # j-replication materialized in SBUF
R = L // J0                # stride-0 repeats per output DMA
E = J0 * DIM               # free elems per (partition, rep)
F32 = mybir.dt.float32

_cache = {}


def _build_nc():
    nc = bass.Bass()
    xs = nc.declare_dram_parameter("xs", [QS, DIM], F32, isOutput=False)
    cs = nc.declare_dram_parameter("consts", [3, DIM], F32, isOutput=False)
    out = nc.declare_dram_parameter("out", [QS, L * DIM], F32, isOutput=True)

    with (
        nc.sbuf_tensor([P, G * DIM], F32) as xsb,     # [p, g*16+d] = x[g*128+p, d]
        nc.sbuf_tensor([P, 3 * DIM], F32) as csb,     # wv | wout | bout, bcast to all p
        nc.sbuf_tensor([P, G * DIM], F32) as prod,    # scratch for accum dot
        nc.sbuf_tensor([P, G], F32) as vsb,           # v[p, g]
        nc.sbuf_tensor([P, E], F32) as rep0,
        nc.sbuf_tensor([P, E], F32) as rep1,
        nc.semaphore() as dsem,
        nc.semaphore() as vsem,
        nc.semaphore() as osem,
        nc.Block(no_gpsimd_drain=True) as block,
    ):
        reps = (rep0, rep1)

        @block.sync
        def _(sync):
            sync.dma_start(
                xsb[:].rearrange("p (g d) -> p g d", d=DIM),
                xs[:].rearrange("(g p) d -> p g d", p=P),
            ).then_inc(dsem, 16)
            for g in range(G):
                sync.wait_ge(vsem, 3 + g)  # rep_g write retired
                sync.dma_start(
                    out[:][g * P:(g + 1) * P, :].rearrange("p (r e) -> p r e", e=E),
                    reps[g][:][:, None, :].broadcast_to((P, R, E)),
                ).then_inc(osem, 16)
            sync.wait_ge(osem, 16 * G)

        @block.scalar
        def _(scalar):
            scalar.dma_start(
                csb[:].rearrange("p (k d) -> p k d", d=DIM),
                cs[:][None, :, :].broadcast_to((P, 3, DIM)),
            ).then_inc(dsem, 16)

        @block.vector
        def _(vector):
            # DVE is deeply pipelined with no SBUF scoreboard: every RAW
            # between instructions needs a semaphore edge, even same-engine.
            vector.wait_ge(dsem, 32)
            wv = csb[:][:, 0:DIM]
            wout = csb[:][:, DIM:2 * DIM]
            bout = csb[:][:, 2 * DIM:3 * DIM]
            # v[p, g] = sum_d x[p, g, d] * wv[d]   (accum_out does the reduce)
            for g in range(G):
                vector.scalar_tensor_tensor(
                    prod[:, g * DIM:(g + 1) * DIM],
                    xsb[:, g * DIM:(g + 1) * DIM],
                    1.0,
                    wv,
                    op0=mybir.AluOpType.mult,
                    op1=mybir.AluOpType.mult,
                    accum_out=vsb[:][:, g:g + 1],
                ).then_inc(vsem, 1)
            vector.wait_ge(vsem, G)
            # rep_g[p, r, d] = wout[d] * v[p, g] + bout[d], all J0 reps at once
            for g in range(G):
                vector.scalar_tensor_tensor(
                    reps[g][:].rearrange("p (r d) -> p r d", d=DIM),
                    wout[:, None, :].broadcast_to((P, J0, DIM)),
                    vsb[:][:, g:g + 1],
                    bout[:, None, :].broadcast_to((P, J0, DIM)),
                    op0=mybir.AluOpType.mult,
                    op1=mybir.AluOpType.add,
                ).then_inc(vsem, 1)

    return nc


def _get_nc():
    if "nc" not in _cache:
        _cache["nc"] = _build_nc()
    return _cache["nc"]


def run(x, W_qkv, W_out, b_out, trace=False):
    nc = _get_nc()
    consts = np.ascontiguousarray(
        np.stack([W_qkv[2, :], W_out[:, 0], b_out]).astype(np.float32)
    )
    in_maps = [
        {
            "xs": np.ascontiguousarray(x[0, c * QS:(c + 1) * QS, :], dtype=np.float32),
            "consts": consts,
        }
        for c in range(N_CORES)
    ]
    res = run_bass_kernel_spmd(nc, in_maps, list(range(N_CORES)), trace=trace)
    shards = [res.results[c]["out"].reshape(QS, L, DIM) for c in range(N_CORES)]
    full = np.concatenate(shards, axis=0)[None]  # (1, 2048, 2048, 16)
    return full, res.exec_time_ns


def kernel(x, W_qkv, W_out, b_out):
    out, _ = run(x, W_qkv, W_out, b_out, trace=False)
    return out


# revision 4
# speedup vs baseline: 1.0216x; 1.0118x over previous
"""Trainium2 Bass kernel for nn_Attention_54717883351680.

Math: with HEADS=1 the softmax in the reference is over a size-1 axis, so
attn == 1 and the whole module collapses to

    out[0, i, j, :] = v[i] * W_out[:, 0] + b_out        (independent of j)
    v[i] = x[0, i, :] @ W_qkv[2, :]

The problem is pure memory bandwidth: write 256 MB of broadcast rows.
Sharding: queries i are split across 8 cores (256 rows each -> 32 MB/core).
Each core computes v for its queries on-chip, writes the replicated row
blocks into two small SBUF tiles (one per 128-query group), and streams its
contiguous 32 MB output shard to DRAM with one large DMA per group whose
source AP uses stride-0 (broadcast) repeats of the tile.
"""

import numpy as np

import concourse.bass as bass
import concourse.mybir as mybir
from concourse.bass_utils import run_bass_kernel_spmd

# Problem shape (hardcoded; kernel.py must be self-contained).
B, L, DIM = 1, 2048, 16
N_CORES = 8
QS = L // N_CORES          # queries per core = 256
P = 128                    # SBUF partitions
G = QS // P                # query groups per core = 2
J0 = 128                   # j-replication materialized in SBUF
R = L // J0                # stride-0 repeats per output DMA
E = J0 * DIM               # free elems per (partition, rep)
F32 = mybir.dt.float32

_cache = {}


def _build_nc():
    nc = bass.Bass()
    xs = nc.declare_dram_parameter("xs", [QS, DIM], F32, isOutput=False)
    cs = nc.declare_dram_parameter("consts", [3, DIM], F32, isOutput=False)
    out = nc.declare_dram_parameter("out", [QS, L * DIM], F32, isOutput=True)

    with (
        nc.sbuf_tensor([P, G * DIM], F32) as xsb,     # [p, g*16+d] = x[g*128+p, d]
        nc.sbuf_tensor([P, 3 * DIM], F32) as csb,     # wv | wout | bout, bcast to all p
        nc.sbuf_tensor([P, G * DIM], F32) as prod,    # scratch for accum dot
        nc.sbuf_tensor([P, G], F32) as vsb,           # v[p, g]
        nc.sbuf_tensor([P, E], F32) as rep0,
        nc.sbuf_tensor([P, E], F32) as rep1,
        nc.semaphore() as dsem,
        nc.semaphore() as vsem,
        nc.semaphore() as osem,
    ):
        reps = (rep0, rep1)
        # Straight-line per-engine emission, no nc.Block: skipping the
        # block-exit all-engine barrier lets each engine reach the NEFF
        # epilogue (per-engine EVSEM resets, ~1-7us) as soon as its own
        # work is done, overlapping it with the 97us output stream.
        nc.sync.dma_start(
            xsb[:].rearrange("p (g d) -> p g d", d=DIM),
            xs[:].rearrange("(g p) d -> p g d", p=P),
        ).then_inc(dsem, 16)
        nc.scalar.dma_start(
            csb[:].rearrange("p (k d) -> p k d", d=DIM),
            cs[:][None, :, :].broadcast_to((P, 3, DIM)),
        ).then_inc(dsem, 16)

        # DVE is deeply pipelined with no SBUF scoreboard: every RAW
        # between instructions needs a semaphore edge, even same-engine.
        nc.vector.wait_ge(dsem, 32)
        wv = csb[:][:, 0:DIM]
        wout = csb[:][:, DIM:2 * DIM]
        bout = csb[:][:, 2 * DIM:3 * DIM]
        # v[p, g] = sum_d x[p, g, d] * wv[d]   (accum_out does the reduce)
        for g in range(G):
            nc.vector.scalar_tensor_tensor(
                prod[:, g * DIM:(g + 1) * DIM],
                xsb[:, g * DIM:(g + 1) * DIM],
                1.0,
                wv,
                op0=mybir.AluOpType.mult,
                op1=mybir.AluOpType.mult,
                accum_out=vsb[:][:, g:g + 1],
            ).then_inc(vsem, 1)
        nc.vector.wait_ge(vsem, G)
        # rep_g[p, r, d] = wout[d] * v[p, g] + bout[d], all J0 reps at once
        for g in range(G):
            nc.vector.scalar_tensor_tensor(
                reps[g][:].rearrange("p (r d) -> p r d", d=DIM),
                wout[:, None, :].broadcast_to((P, J0, DIM)),
                vsb[:][:, g:g + 1],
                bout[:, None, :].broadcast_to((P, J0, DIM)),
                op0=mybir.AluOpType.mult,
                op1=mybir.AluOpType.add,
            ).then_inc(vsem, 1)

        for g in range(G):
            nc.sync.wait_ge(vsem, 3 + g)  # rep_g write retired
            nc.sync.dma_start(
                out[:][g * P:(g + 1) * P, :].rearrange("p (r e) -> p r e", e=E),
                reps[g][:][:, None, :].broadcast_to((P, R, E)),
            ).then_inc(osem, 16)
        nc.sync.wait_ge(osem, 16 * G)

    return nc


def _get_nc():
    if "nc" not in _cache:
        _cache["nc"] = _build_nc()
    return _cache["nc"]


def run(x, W_qkv, W_out, b_out, trace=False):
    nc = _get_nc()
    consts = np.ascontiguousarray(
        np.stack([W_qkv[2, :], W_out[:, 0], b_out]).astype(np.float32)
    )
    in_maps = [
        {
            "xs": np.ascontiguousarray(x[0, c * QS:(c + 1) * QS, :], dtype=np.float32),
            "consts": consts,
        }
        for c in range(N_CORES)
    ]
    res = run_bass_kernel_spmd(nc, in_maps, list(range(N_CORES)), trace=trace)
    shards = [res.results[c]["out"].reshape(QS, L, DIM) for c in range(N_CORES)]
    full = np.concatenate(shards, axis=0)[None]  # (1, 2048, 2048, 16)
    return full, res.exec_time_ns


def kernel(x, W_qkv, W_out, b_out):
    out, _ = run(x, W_qkv, W_out, b_out, trace=False)
    return out


# revision 5
# speedup vs baseline: 1.0454x; 1.0232x over previous
"""Trainium2 Bass kernel for nn_Attention_54717883351680.

Math: with HEADS=1 the softmax in the reference is over a size-1 axis, so
attn == 1 and the whole module collapses to

    out[0, i, j, :] = v[i] * W_out[:, 0] + b_out        (independent of j)
    v[i] = x[0, i, :] @ W_qkv[2, :]

The problem is pure memory bandwidth: write 256 MB of broadcast rows.
Sharding: queries i are split across 8 cores (256 rows each -> 32 MB/core).
Each core computes v for its queries on-chip, writes the replicated row
blocks into two small SBUF tiles (one per 128-query group), and streams its
contiguous 32 MB output shard to DRAM with one large DMA per group whose
source AP uses stride-0 (broadcast) repeats of the tile.
"""

import numpy as np

import concourse.bass as bass
import concourse.mybir as mybir
from concourse.bass_utils import run_bass_kernel_spmd

# The NEFF epilogue restores every semaphore in the declared kernel range on
# every engine (2 sems per EventSemaphore op, ~115ns each on the PE
# sequencer): the default range(150, 256) costs ~6us of tail. This kernel
# uses <12 sems, so narrow the range.
bass.get_kernel_semaphore_range = lambda: range(150, 170)

# Problem shape (hardcoded; kernel.py must be self-contained).
B, L, DIM = 1, 2048, 16
N_CORES = 8
QS = L // N_CORES          # queries per core = 256
P = 128                    # SBUF partitions
G = QS // P                # query groups per core = 2
J0 = 128                   # j-replication materialized in SBUF
R = L // J0                # stride-0 repeats per output DMA
E = J0 * DIM               # free elems per (partition, rep)
F32 = mybir.dt.float32

_cache = {}


def _build_nc():
    nc = bass.Bass()
    xs = nc.declare_dram_parameter("xs", [QS, DIM], F32, isOutput=False)
    cs = nc.declare_dram_parameter("consts", [3, DIM], F32, isOutput=False)
    out = nc.declare_dram_parameter("out", [QS, L * DIM], F32, isOutput=True)

    with (
        nc.sbuf_tensor([P, G * DIM], F32) as xsb,     # [p, g*16+d] = x[g*128+p, d]
        nc.sbuf_tensor([P, 3 * DIM], F32) as csb,     # wv | wout | bout, bcast to all p
        nc.sbuf_tensor([P, G * DIM], F32) as prod,    # scratch for accum dot
        nc.sbuf_tensor([P, G], F32) as vsb,           # v[p, g]
        nc.sbuf_tensor([P, E], F32) as rep0,
        nc.sbuf_tensor([P, E], F32) as rep1,
        nc.semaphore() as dsem,
        nc.semaphore() as vsem,
        nc.semaphore() as osem,
    ):
        reps = (rep0, rep1)
        # Straight-line per-engine emission, no nc.Block: skipping the
        # block-exit all-engine barrier lets each engine reach the NEFF
        # epilogue (per-engine EVSEM resets, ~1-7us) as soon as its own
        # work is done, overlapping it with the 97us output stream.
        nc.sync.dma_start(
            xsb[:].rearrange("p (g d) -> p g d", d=DIM),
            xs[:].rearrange("(g p) d -> p g d", p=P),
        ).then_inc(dsem, 16)
        nc.scalar.dma_start(
            csb[:].rearrange("p (k d) -> p k d", d=DIM),
            cs[:][None, :, :].broadcast_to((P, 3, DIM)),
        ).then_inc(dsem, 16)

        # DVE is deeply pipelined with no SBUF scoreboard: every RAW
        # between instructions needs a semaphore edge, even same-engine.
        nc.vector.wait_ge(dsem, 32)
        wv = csb[:][:, 0:DIM]
        wout = csb[:][:, DIM:2 * DIM]
        bout = csb[:][:, 2 * DIM:3 * DIM]
        # v[p, g] = sum_d x[p, g, d] * wv[d]   (accum_out does the reduce)
        for g in range(G):
            nc.vector.scalar_tensor_tensor(
                prod[:, g * DIM:(g + 1) * DIM],
                xsb[:, g * DIM:(g + 1) * DIM],
                1.0,
                wv,
                op0=mybir.AluOpType.mult,
                op1=mybir.AluOpType.mult,
                accum_out=vsb[:][:, g:g + 1],
            ).then_inc(vsem, 1)
        nc.vector.wait_ge(vsem, G)
        # rep_g[p, r, d] = wout[d] * v[p, g] + bout[d], all J0 reps at once
        for g in range(G):
            nc.vector.scalar_tensor_tensor(
                reps[g][:].rearrange("p (r d) -> p r d", d=DIM),
                wout[:, None, :].broadcast_to((P, J0, DIM)),
                vsb[:][:, g:g + 1],
                bout[:, None, :].broadcast_to((P, J0, DIM)),
                op0=mybir.AluOpType.mult,
                op1=mybir.AluOpType.add,
            ).then_inc(vsem, 1)

        for g in range(G):
            nc.sync.wait_ge(vsem, 3 + g)  # rep_g write retired
            nc.sync.dma_start(
                out[:][g * P:(g + 1) * P, :].rearrange("p (r e) -> p r e", e=E),
                reps[g][:][:, None, :].broadcast_to((P, R, E)),
            ).then_inc(osem, 16)
        nc.sync.wait_ge(osem, 16 * G)

    return nc


def _get_nc():
    if "nc" not in _cache:
        _cache["nc"] = _build_nc()
    return _cache["nc"]


def run(x, W_qkv, W_out, b_out, trace=False):
    nc = _get_nc()
    consts = np.ascontiguousarray(
        np.stack([W_qkv[2, :], W_out[:, 0], b_out]).astype(np.float32)
    )
    in_maps = [
        {
            "xs": np.ascontiguousarray(x[0, c * QS:(c + 1) * QS, :], dtype=np.float32),
            "consts": consts,
        }
        for c in range(N_CORES)
    ]
    res = run_bass_kernel_spmd(nc, in_maps, list(range(N_CORES)), trace=trace)
    shards = [res.results[c]["out"].reshape(QS, L, DIM) for c in range(N_CORES)]
    full = np.concatenate(shards, axis=0)[None]  # (1, 2048, 2048, 16)
    return full, res.exec_time_ns


def kernel(x, W_qkv, W_out, b_out):
    out, _ = run(x, W_qkv, W_out, b_out, trace=False)
    return out
